# revision 14
# baseline (speedup 1.0000x reference)
"""Trainium2 Bass kernel for nn_BoundarySeg (segment_reduce).

out[b, j, 0:H]   = sum_{i>=j} A[b, j, i] * h[b, i, :]
out[b, j, H:2H]  = h[b, j, :] * sum_{i>=j} A[b, j, i]

Shapes: A [8, 2048, 2048] f32, h [8, 2048, 256] f32 -> out [8, 2048, 512] f32.

Sharding: data-parallel over batch; core c computes batch c.

Per-core algorithm (L=2048 in 16 tiles of 128, H=256):
  - Load h once into SBUF as [128(p), 16(t), 257] with a ones column
    appended at index 256 (so the row-sum of A falls out of the same
    matmul as an extra output column).
  - For each j-tile jc: DMA only the upper panel A[jc, jc:], mask the
    diagonal 128x128 block with an upper-triangular mask, transpose each
    128x128 block on TensorE (PSUM bounce), then accumulate
    acc[j, n] += At[i, j]^T @ h_ext[i, n] over i-tiles ic >= jc.
    Column 256 of acc is the masked row-sum; second half of the output
    is h[j, :] * rowsum[j] via a per-partition tensor_scalar multiply.
  - Matmuls run as float32r (full-rate fp32 mode, N=257 >= 256).

Lower-triangular blocks are never loaded nor computed (halves DMA+FLOPs).
"""

import os
import sys

import numpy as np

sys.path.insert(0, "/opt/trn_rl_repo")

import concourse.bass as bass  # noqa: E402
import concourse.bacc as bacc  # noqa: E402
import concourse.tile as tile  # noqa: E402
from concourse import mybir  # noqa: E402
from concourse.bass_utils import run_bass_kernel_spmd  # noqa: E402
from concourse.masks import make_identity, make_lower_triangular  # noqa: E402

B, L, H = 8, 2048, 256
P = 128
GROUP = 4  # 128-col transposes batched per PSUM tile / DVE copy

DT = mybir.dt.float32

# Results of the last run (exec_time_ns etc.) for the test harness.
LAST_RESULTS = None
_NC_CACHE = {}


def _build_nc(L=L, H=H, mm_dtype=mybir.dt.float32r):
    NT = L // P
    HE = H + 2  # f32r matmul needs even N; col H = ones (rowsum), col H+1 unused
    f32r = mm_dtype

    nc = bacc.Bacc(None, target_bir_lowering=False)
    a_dram = nc.dram_tensor("a", [L, L], DT, kind="ExternalInput")
    h_dram = nc.dram_tensor("h", [L, H], DT, kind="ExternalInput")
    out_dram = nc.dram_tensor("out", [L, 2 * H], DT, kind="ExternalOutput")

    with tile.TileContext(nc) as tc:
        with (
            tc.tile_pool(name="const", bufs=1) as const_pool,
            tc.tile_pool(name="hpool", bufs=1) as h_pool,
            tc.tile_pool(name="apanel", bufs=3) as a_pool,
            tc.tile_pool(name="atT", bufs=2) as at_pool,
            tc.tile_pool(name="tp", bufs=3, space=bass.MemorySpace.PSUM) as tp_pool,
            tc.tile_pool(name="acc", bufs=2, space=bass.MemorySpace.PSUM) as acc_pool,
            tc.tile_pool(name="outsb", bufs=2) as out_pool,
            tc.tile_pool(name="small", bufs=2) as small_pool,
        ):
            identity = const_pool.tile([P, P], DT)
            make_identity(nc, identity[:])
            # Mask applied to the *transposed* diagonal block during the
            # PSUM->SBUF copy: block is [i(part), j(free)], keep i >= j
            # (lower triangular). Columns P..GROUP*P multiply by 1.0.
            # Built on gpsimd, then bounced through a DVE copy so consumers
            # of cmask depend on DVE (same engine) instead of Pool — walrus
            # allows very few sync waits per compute instruction.
            mask_src = const_pool.tile([P, P], DT)
            make_lower_triangular(nc, mask_src[:], val=1.0, diag=True)
            cmask = const_pool.tile([P, GROUP * P], DT)
            nc.vector.tensor_copy(cmask[:, 0:P], mask_src[:])
            nc.vector.memset(cmask[:, P : GROUP * P], 1.0)

            # h_stage: exact fp32 h plus a ones column at [.., H]; h_all: the
            # f32r-rounded copy (walrus requires f32r matmul operands to come
            # from an instruction that rounds to f32r, and memset cannot
            # target f32r, hence staging + one DVE cast-copy).
            h_stage = h_pool.tile([P, NT, HE], DT)
            nc.sync.dma_start(
                out=h_stage[:, :, 0:H],
                in_=h_dram[:].rearrange("(t p) n -> p t n", p=P),
            )
            nc.vector.memset(h_stage[:, :, H:HE], 1.0)
            h_all = h_pool.tile([P, NT, HE], f32r)
            nc.vector.tensor_copy(h_all[:], h_stage[:])

            # Warmup transpose: absorbs the Pool->PE wait for `identity` so
            # the first real transpose of each panel carries only its DMA wait.
            wtp = tp_pool.tile([P, GROUP * P], DT, tag="tp")
            nc.tensor.transpose(wtp[:, 0:P], identity[:], identity[:])

            for jc in range(NT):
                ntiles = NT - jc
                W = ntiles * P
                a_panel = a_pool.tile([P, W], DT, tag="apanel")
                nc.sync.dma_start(
                    a_panel[:], a_dram[jc * P : (jc + 1) * P, jc * P : L]
                )

                # transpose the whole panel: at[:, k*P:(k+1)*P] = A-block(jc, jc+k).T
                # (fp32 transpose through PSUM; the PSUM->SBUF copy rounds to
                # f32r and applies the diagonal mask on group 0)
                atT = at_pool.tile([P, W], f32r, tag="atT")
                for g0 in range(0, ntiles, GROUP):
                    gn = min(GROUP, ntiles - g0)
                    tp = tp_pool.tile([P, GROUP * P], DT, tag="tp")
                    for k in range(gn):
                        nc.tensor.transpose(
                            tp[:, k * P : (k + 1) * P],
                            a_panel[:, (g0 + k) * P : (g0 + k + 1) * P],
                            identity[:],
                        )
                    if g0 == 0:
                        nc.vector.tensor_tensor(
                            atT[:, 0 : gn * P],
                            tp[:, 0 : gn * P],
                            cmask[:, 0 : gn * P],
                            mybir.AluOpType.mult,
                        )
                    else:
                        nc.vector.tensor_copy(
                            atT[:, g0 * P : (g0 + gn) * P], tp[:, 0 : gn * P]
                        )

                acc = acc_pool.tile([P, HE], DT, tag="acc")
                for k in range(ntiles):
                    ic = jc + k
                    nc.tensor.matmul(
                        acc[:],
                        atT[:, k * P : (k + 1) * P],
                        h_all[:, ic, :],
                        start=(k == 0),
                        stop=(k == ntiles - 1),
                    )

                out_sb = out_pool.tile([P, 2 * H], DT, tag="outsb")
                rowsum = small_pool.tile([P, 1], DT, tag="rowsum")
                nc.vector.tensor_copy(rowsum[:], acc[:, H : H + 1])
                nc.vector.tensor_copy(out_sb[:, 0:H], acc[:, 0:H])
                nc.vector.tensor_scalar_mul(
                    out_sb[:, H : 2 * H], h_stage[:, jc, 0:H], rowsum[:]
                )
                nc.sync.dma_start(out_dram[jc * P : (jc + 1) * P, :], out_sb[:])

    nc.finalize()
    return nc


def kernel(span_adjacency, bound_hidden):
    global LAST_RESULTS
    a = np.ascontiguousarray(np.asarray(span_adjacency, dtype=np.float32))
    h = np.ascontiguousarray(np.asarray(bound_hidden, dtype=np.float32))
    assert a.shape == (B, L, L) and h.shape == (B, L, H), (a.shape, h.shape)

    key = "full"
    if key not in _NC_CACHE:
        _NC_CACHE[key] = _build_nc()
    nc = _NC_CACHE[key]

    in_maps = [{"a": a[b], "h": h[b]} for b in range(B)]
    res = run_bass_kernel_spmd(
        nc,
        in_maps,
        core_ids=list(range(B)),
        trace=bool(os.environ.get("KERNEL_TRACE")),
    )
    LAST_RESULTS = res
    out = np.stack([res.results[b]["out"] for b in range(B)], axis=0)
    return out
